# revision 25
# baseline (speedup 1.0000x reference)
"""CrossAttention kernel for 8 Trainium2 NeuronCores.

Problem: x,y [4,2048,64] f32, mask [1,2048,2048] i32, per-head projections
Wk/Wq/Wv [64, 512] (8 heads x head_dim 64), unify Wu [512,64] + bu.

Sharding: split the query axis t_y across the 8 cores (256 queries each, for
all batches/heads). Each core reads the full x (2MB), its y slice, and its
256x2048 mask slice; writes a disjoint out slice. No collectives needed.

Per-core pipeline (v2):
- scores S^T [k, q] via folded (Wk_h^T Wq_h) weight products, f32r matmuls.
- PSUM->SBUF evacuation of exp(S)*mask is split three ways per (b,h):
  'A': ACT exp (+ DVE bf16 mask multiply),
  'D': DVE fused scalar_tensor_tensor Schraudolph (int16 out, bitcast bf16;
       mask folded additively into the magic-constant tensor),
  'P': same fused op on GPSIMD.
- attention*V consumes att as the stationary operand producing O' [q, 65]
  with full 128 output partitions (half the PE streaming of the [65, q]
  orientation); col 64 is the ones-column softmax denominator, so the
  normalization is a per-partition reciprocal + scaled copy.
- O'n [q, e] is PE-transposed back to [e, q] (Pool evacuates) to feed the
  unify GEMM sum_h O'n_h^T (Wv_h Wu_h) + bu.
"""

import numpy as np

import bass_rust
import concourse.bass as bass
import concourse.mybir as mybir
import concourse.tile as tile
from bass_rust import ScopedClock, SemaphoreHandle
from concourse.bass_utils import run_bass_kernel_spmd
from concourse.masks import make_identity

# ---------------------------------------------------------------------------
# Workaround for walrus codegen "Too many sync wait commands" on the
# TileContext tail drain: the CoreV3 CTRL encoding takes one sync wait, so
# replay the drain's wait set as standalone single-wait SP instructions.
# ---------------------------------------------------------------------------


def _drain_and_barrier_split(self, tick_clock, wait_clock):
    nc = self.nc
    probe = nc.sync.nop()
    wait_clock.add_sem_waits(probe.ins, ScopedClock({None: tick_clock.global_clock}))
    si = probe.ins.sync_info
    waits = list(si.on_wait or []) if si is not None else []
    if si is not None:
        si.on_wait = []
        probe.ins.sync_info = si
    for w in waits:
        op = {"sem-ge-imm": "sem-ge", "sem-eq-imm": "sem-eq"}.get(w.wait_mode, "sem-ge")
        nc.sync.wait_op(SemaphoreHandle(w.ant_name or "w", w.id), w.wait_value, op)
    nc.sync.drain()

    nc.all_engine_barrier()
    assert self.sems is not None
    popped = nc._tile_sem_poison_stack.pop()
    assert popped is self._sem_poison
    nc.clear_and_free_semaphores(list(self.sems.allocated().values()))
    nc.all_engine_barrier()


tile.TileContext._drain_and_barrier = _drain_and_barrier_split


def legalize_waits(nc, max_waits=1):
    """Walrus's ISA structs encode at most one sync wait per instruction.
    Hoist extra waits onto standalone same-engine NOPs inserted right
    before the over-subscribed instruction (identical blocking semantics)."""
    cur_list = nc.cur_bb.bb.instructions
    for bb in nc.m.functions[0].blocks:
        insts = bb.instructions
        i = 0
        while i < len(insts):
            ins = insts[i]
            si = getattr(ins, "sync_info", None)
            waits = list(si.on_wait or []) if si is not None else []
            movable = [w for w in waits if w.wait_reg is None]
            if len(waits) > max_waits and len(movable) > len(waits) - max_waits:
                nkeep = max_waits
                extra = movable[: len(waits) - nkeep]
                extra_set = {id(w) for w in extra}
                si.on_wait = [w for w in waits if id(w) not in extra_set]
                ins.sync_info = si
                carriers = []
                for w in extra:
                    nop = nc.engines[ins.engine].nop().ins
                    # the builder appended it to cur_bb; move it here
                    popped = cur_list.pop()
                    assert popped is nop
                    nop.sync_info = bass_rust.SyncInfo(on_wait=[w], on_update=[])
                    carriers.append(nop)
                insts[i:i] = carriers
                i += len(carriers)
            i += 1

# ---------------------------------------------------------------------------

B, T, E, H = 4, 2048, 64, 8
NCORES = 8
QS = T // NCORES          # 256 queries per core
NT = B * T // 128         # 64 token tiles of 128
KTB = T // 128            # 16 key tiles per batch
SCALE = 1.0 / np.sqrt(E)  # folded into exp()

# bf16 Schraudolph: int16 = round(s * C16 + MBD), bitcast to bf16 ~ exp(s*SCALE)*m
C16 = 128.0 * SCALE / float(np.log(2.0))   # 23.0831...
MAGIC = 16247.0                            # calibrated on-device (round-to-nearest)
MASK16 = 5504.0                            # 43*128: masked entries -> ~2^-43
NORM_ON_ACT = True

F32 = mybir.dt.float32
F32R = mybir.dt.float32r
BF16 = mybir.dt.bfloat16
I16 = mybir.dt.int16
I32 = mybir.dt.int32

Exp = mybir.ActivationFunctionType.Exp
Mult = mybir.AluOpType.mult
Add = mybir.AluOpType.add

# evacuation engine per score group: 'A' ACT-exp, 'D' DVE-fused
# (walrus rejects scalar_tensor_tensor on Pool, so Pool only does copies)
GROUP_PATHS = ["A", "D", "A", "D", "A", "D", "A", "A"]


def build():
    nc = bass.Bass()
    x_d = nc.dram_tensor("x", [B * T, E], F32, kind="ExternalInput")
    y_d = nc.dram_tensor("ysl", [B * QS, E], F32, kind="ExternalInput")
    m_d = nc.dram_tensor("masksl", [QS, T], I32, kind="ExternalInput")
    wk_d = nc.dram_tensor("Wk", [E, E * H], F32, kind="ExternalInput")
    wq_d = nc.dram_tensor("Wq", [E, E * H], F32, kind="ExternalInput")
    wv_d = nc.dram_tensor("Wv", [E, E * H], F32, kind="ExternalInput")
    wu_d = nc.dram_tensor("Wu", [E * H, E], F32, kind="ExternalInput")
    bu_d = nc.dram_tensor("bu", [1, E], F32, kind="ExternalInput")
    out_d = nc.dram_tensor("out", [B * QS, E], F32, kind="ExternalOutput")

    with tile.TileContext(nc) as tc:
        with (
            tc.tile_pool(name="const", bufs=1) as cp,
            tc.tile_pool(name="big", bufs=1) as bigp,
            tc.tile_pool(name="att", bufs=3) as attp,
            tc.tile_pool(name="small", bufs=3) as smallp,
            tc.tile_pool(name="qp", bufs=2) as qpool,
            tc.tile_pool(name="ps512", bufs=4, space="PSUM") as pss,
            tc.tile_pool(name="pso", bufs=4, space="PSUM") as pso,
        ):
            # ---- constants ----
            ident = cp.tile([128, 128], F32)
            make_identity(nc, ident[:])
            identb = cp.tile([128, 128], BF16)
            nc.vector.tensor_copy(identb[:], ident[:])
            wk_t = cp.tile([64, 512], F32)
            nc.sync.dma_start(wk_t[:], wk_d[:])
            wq_t = cp.tile([64, 512], F32)
            nc.sync.dma_start(wq_t[:], wq_d[:])
            wv_t = cp.tile([64, 512], F32)
            wu8 = cp.tile([64, H, 64], F32)
            bu_f = cp.tile([1, 64], F32)
            bub = cp.tile([1, 64], BF16)
            ones_r = cp.tile([1, 128], BF16)   # bias broadcast lhsT
            nc.gpsimd.memset(ones_r[:], 1.0)

            # ---- persistent big tiles ----
            xT = bigp.tile([64, B * T], F32R)     # x^T [e_in, tokens]
            yT = bigp.tile([64, B * QS], F32R)    # y^T [e_in, queries]
            mT = bigp.tile([128, KTB, QS], BF16)  # mask^T 0/1 [k, q]
            mbd = bigp.tile([128, KTB, QS], F32)  # Schraudolph magic+mask [k, q]
            Xb = bigp.tile([128, NT, 65], BF16)   # X|1 per token tile
            W3T = bigp.tile([64, 4, 128], F32R)   # (Wk_h Wq_h^T)^T head pairs
            M2 = bigp.tile([64, H, 64], BF16)     # Wv_h @ Wu_h
            Obn = bigp.tile([128, 2, B, H, 64], F32)  # normalized O' [q, e]
            outs = bigp.tile([128, 8, 64], F32)   # final output staging

            nc.gpsimd.memset(Xb[:, :, 64:65], 1.0)  # ones column

            # ---- setup: transposes, weight products ----
            with (
                tc.tile_pool(name="ldx", bufs=1) as ldp,
                tc.tile_pool(name="ldx2", bufs=3) as ldp2,
                tc.tile_pool(name="ldm", bufs=1) as lmp,
            ):
                # y^T from one staged load
                ys = ldp.tile([128, B * QS // 128, 64], F32, tag="ys")
                nc.sync.dma_start(
                    ys[:], y_d[:].rearrange("(n p) e -> p n e", p=128)
                )
                for i in range(B * QS // 128):
                    pt = pso.tile([128, 128], F32, tag="pO")
                    nc.tensor.transpose(pt[:64, :], ys[:, i, :], ident[:])
                    nc.vector.tensor_copy(yT[:, i * 128:(i + 1) * 128], pt[:64, :])

                # per-head weight transposes (f32, tiny)
                wkT = cp.tile([64, H, 64], F32)
                wqT = cp.tile([64, H, 64], F32)
                wvT = cp.tile([64, H, 64], F32)
                for h in range(H):
                    for wsrc, wdst in ((wk_t, wkT), (wq_t, wqT)):
                        pt = pso.tile([128, 128], F32, tag="pO")
                        nc.tensor.transpose(
                            pt[:64, 0:64], wsrc[:, h * 64:(h + 1) * 64],
                            ident[:64, :64]
                        )
                        nc.vector.tensor_copy(wdst[:, h, :], pt[:64, 0:64])
                # W3_h^T = Wq_h^T . (Wk_h^T as rhs): out [ein_q, ein_k]
                for h in range(H):
                    pt = pso.tile([128, 128], F32, tag="pO")
                    nc.tensor.matmul(
                        pt[:64, 0:64], wqT[:, h, :], wkT[:, h, :],
                        start=True, stop=True
                    )
                    nc.vector.tensor_copy(
                        W3T[:, h // 2, (h % 2) * 64:(h % 2) * 64 + 64],
                        pt[:64, 0:64]
                    )
                # mask: int32 -> f32 -> transpose -> bf16 m^T (0/1)
                for qt in range(QS // 128):
                    mi = lmp.tile([128, T], I32, tag="mi")
                    for mh in range(2):
                        nc.sync.dma_start(
                            mi[:, mh * 1024:(mh + 1) * 1024],
                            m_d[qt * 128:(qt + 1) * 128,
                                mh * 1024:(mh + 1) * 1024],
                        )
                    mf = lmp.tile([128, T], F32, tag="mf")
                    for mh in range(2):
                        nc.gpsimd.tensor_copy(
                            mf[:, mh * 1024:(mh + 1) * 1024],
                            mi[:, mh * 1024:(mh + 1) * 1024],
                        )
                    for ki in range(KTB):
                        pt = pso.tile([128, 128], F32, tag="pO")
                        nc.tensor.transpose(
                            pt[:], mf[:, ki * 128:(ki + 1) * 128], ident[:]
                        )
                        nc.scalar.copy(mT[:, ki, qt * 128:(qt + 1) * 128], pt[:])
                # Schraudolph magic tensor: MAGIC - MASK16*(1-m)
                for half in range(2):
                    nc.vector.tensor_scalar(
                        mbd[:, half * 8:(half + 1) * 8, :],
                        mT[:, half * 8:(half + 1) * 8, :],
                        MASK16, MAGIC - MASK16, Mult, Add,
                    )

                # x^T (f32r) and X|1 (bf16), streamed in 8 chunks
                NX = 8
                for c in range(NT // NX):
                    xs = ldp2.tile([128, NX, 64], F32, tag="xs")
                    nc.sync.dma_start(
                        xs[:],
                        x_d[c * NX * 128:(c + 1) * NX * 128, :].rearrange(
                            "(n p) e -> p n e", p=128
                        ),
                    )
                    for j in range(NX):
                        i = c * NX + j
                        pt = pso.tile([128, 128], F32, tag="pO")
                        nc.tensor.transpose(pt[:64, :], xs[:, j, :], ident[:])
                        nc.vector.tensor_copy(
                            xT[:, i * 128:(i + 1) * 128], pt[:64, :]
                        )
                        nc.gpsimd.tensor_copy(Xb[:, i, 0:64], xs[:, j, :])

                # late-needed weights (M2 inputs + bias)
                nc.sync.dma_start(wv_t[:], wv_d[:])
                for h in range(H):
                    nc.sync.dma_start(wu8[:, h, :], wu_d[h * 64:(h + 1) * 64, :])
                nc.sync.dma_start(bu_f[:], bu_d[:])
                nc.vector.tensor_copy(bub[:], bu_f[:])

                for h in range(H):
                    pt = pso.tile([128, 128], F32, tag="pO")
                    nc.tensor.transpose(
                        pt[:64, 0:64], wv_t[:, h * 64:(h + 1) * 64],
                        ident[:64, :64]
                    )
                    nc.vector.tensor_copy(wvT[:, h, :], pt[:64, 0:64])
                # M2_h = Wv_h @ Wu_h
                for h in range(H):
                    pt = pso.tile([128, 128], F32, tag="pO")
                    nc.tensor.matmul(
                        pt[:64, 0:64], wvT[:, h, :], wu8[:, h, :],
                        start=True, stop=True,
                    )
                    nc.vector.tensor_copy(M2[:, h, :], pt[:64, 0:64])

            # ---- main loop: software-pipelined over (b, h) ----
            # iteration i emits scores+evac(i), then AV+normalize(i-1), so
            # each in-order engine queue always has ready work at its head.

            def emit_scores_evac(hp, b, h, Qp):
                att = attp.tile([128, KTB, QS], BF16, tag="att")
                for kj in range(KTB // 2):
                    gt = b * KTB + 2 * kj
                    pS = pss.tile([128, 512], F32, tag="ps512")
                    for u in range(2):
                        nc.tensor.matmul(
                            pS[:, u * 256:(u + 1) * 256],
                            xT[:, (gt + u) * 128:(gt + u + 1) * 128],
                            Qp[:, b * QS:(b + 1) * QS],
                            start=True, stop=True,
                        )
                    sl = att[:, 2 * kj:2 * kj + 2, :]
                    path = GROUP_PATHS[kj]
                    if path == "A":
                        nc.scalar.activation(sl, pS[:], Exp, scale=SCALE)
                        nc.vector.tensor_mul(
                            sl, sl, mT[:, 2 * kj:2 * kj + 2, :]
                        )
                    elif path == "D":
                        nc.vector.scalar_tensor_tensor(
                            sl.bitcast(I16), pS[:], C16,
                            mbd[:, 2 * kj:2 * kj + 2, :],
                            op0=Mult, op1=Add,
                        )
                return att

            def emit_av_norm(b, h, attB):
                for qt in range(2):
                    pO = pso.tile([128, 128], F32, tag="pO")
                    for ki in range(KTB):
                        nc.tensor.matmul(
                            pO[:, 0:65],
                            attB[:, ki, qt * 128:(qt + 1) * 128],
                            Xb[:, b * KTB + ki, :],
                            start=(ki == 0), stop=(ki == KTB - 1),
                            skip_group_check=True,
                        )
                    rd = smallp.tile([128, 1], F32, tag="rd")
                    nc.vector.reciprocal(rd[:], pO[:, 64:65])
                    obn = Obn[:, qt, b, h, :]
                    if NORM_ON_ACT:
                        nc.scalar.mul(obn, pO[:, 0:64], rd[:])
                    else:
                        nc.vector.tensor_scalar(
                            obn, pO[:, 0:64], rd[:], None, Mult
                        )

            prev = None
            for hp in range(4):
                # Q'^T for the pair: rows 0:64 = h0, 64:128 = h1
                QpT0 = qpool.tile([64, B * QS], F32R, tag="qp0")
                QpT1 = qpool.tile([64, B * QS], F32R, tag="qp1")
                for i in range(B * QS // 512):
                    pq = pss.tile([128, 512], F32, tag="ps512")
                    nc.tensor.matmul(
                        pq[:], W3T[:, hp, :],
                        yT[:, i * 512:(i + 1) * 512],
                        start=True, stop=True,
                    )
                    nc.vector.tensor_copy(
                        QpT0[:, i * 512:(i + 1) * 512], pq[0:64, :]
                    )
                    nc.scalar.copy(
                        QpT1[:, i * 512:(i + 1) * 512], pq[64:128, :]
                    )

                for b in range(B):
                    for hh in range(2):
                        h = 2 * hp + hh
                        Qp = QpT0 if hh == 0 else QpT1
                        attB = emit_scores_evac(hp, b, h, Qp)
                        if prev is not None:
                            emit_av_norm(*prev)
                        prev = (b, h, attB)
            emit_av_norm(*prev)

            # ---- phase 2: transpose O'n, then unify ----
            ObnT = bigp.tile([64, B, H, QS], BF16)  # O'n^T [e, q]
            for i, (b, h, qt) in enumerate(
                (b, h, qt) for b in range(B) for h in range(H)
                for qt in range(2)
            ):
                pT = pso.tile([128, 128], F32, tag="pO")
                nc.tensor.transpose(pT[:64, :], Obn[:, qt, b, h, :], ident[:])
                dst = ObnT[:, b, h, qt * 128:(qt + 1) * 128]
                if i % 2 == 0:
                    nc.vector.tensor_copy(dst, pT[:64, :])
                else:
                    nc.scalar.copy(dst, pT[:64, :])

            # ---- final: out = sum_h O'n_h^T M2_h + bu ----
            for b in range(B):
                for qt in range(QS // 128):
                    pU = pso.tile([128, 128], F32, tag="pO")
                    for h in range(H):
                        nc.tensor.matmul(
                            pU[:, 0:64],
                            ObnT[:, b, h, qt * 128:(qt + 1) * 128],
                            M2[:, h, :],
                            start=(h == 0), stop=False,
                            skip_group_check=True,
                        )
                    nc.tensor.matmul(
                        pU[:, 0:64], ones_r[:], bub[:],
                        start=False, stop=True,
                        skip_group_check=True,
                    )
                    nc.scalar.copy(outs[:, b * 2 + qt, :], pU[:, 0:64])
                nc.sync.dma_start(
                    out_d[:].rearrange("(s p) e -> p s e", p=128), outs[:]
                )
    legalize_waits(nc)
    return nc


_NC = None


def _get_nc():
    global _NC
    if _NC is None:
        _NC = build()
    return _NC


LAST_EXEC_NS = None
LAST_RESULTS = None


def kernel(x, y, mask, Wk, Wq, Wv, Wu, bu, trace=False):
    global LAST_EXEC_NS, LAST_RESULTS
    x = np.ascontiguousarray(np.asarray(x, dtype=np.float32)).reshape(B * T, E)
    y = np.ascontiguousarray(np.asarray(y, dtype=np.float32))
    mask = np.ascontiguousarray(np.asarray(mask, dtype=np.int32))
    Wk = np.ascontiguousarray(np.asarray(Wk, dtype=np.float32))
    Wq = np.ascontiguousarray(np.asarray(Wq, dtype=np.float32))
    Wv = np.ascontiguousarray(np.asarray(Wv, dtype=np.float32))
    Wu = np.ascontiguousarray(np.asarray(Wu, dtype=np.float32))
    bu = np.ascontiguousarray(np.asarray(bu, dtype=np.float32)).reshape(1, E)

    nc = _get_nc()
    in_maps = []
    for c in range(NCORES):
        q0 = c * QS
        in_maps.append({
            "x": x,
            "ysl": np.ascontiguousarray(y[:, q0:q0 + QS, :]).reshape(B * QS, E),
            "masksl": np.ascontiguousarray(mask[0, q0:q0 + QS, :]),
            "Wk": Wk, "Wq": Wq, "Wv": Wv, "Wu": Wu, "bu": bu,
        })
    res = run_bass_kernel_spmd(
        nc, in_maps, core_ids=list(range(NCORES)), trace=trace
    )
    LAST_EXEC_NS = res.exec_time_ns
    LAST_RESULTS = res
    out = np.empty((B, T, E), dtype=np.float32)
    for c in range(NCORES):
        q0 = c * QS
        out[:, q0:q0 + QS, :] = res.results[c]["out"].reshape(B, QS, E)
    return out


# revision 26
# speedup vs baseline: 1.0215x; 1.0215x over previous
"""CrossAttention kernel for 8 Trainium2 NeuronCores.

Problem: x,y [4,2048,64] f32, mask [1,2048,2048] i32, per-head projections
Wk/Wq/Wv [64, 512] (8 heads x head_dim 64), unify Wu [512,64] + bu.

Sharding: split the query axis t_y across the 8 cores (256 queries each, for
all batches/heads). Each core reads the full x (2MB), its y slice, and its
256x2048 mask slice; writes a disjoint out slice. No collectives needed.

Per-core layout: scores are computed transposed (S^T [k, q]) so that the
attention*V matmul can consume them directly as the moving operand with V as
the stationary operand, producing O^T [e, q] chunks that feed the unify GEMM
as its stationary operand with no further transposes. Softmax over k (the
partition dim of S^T) needs no max-subtraction (|S|<~1 by construction); the
denominator comes for free from a ones-column appended to V, and the
binary mask is applied multiplicatively to exp(S^T) on the vector engine.
"""

import numpy as np

import bass_rust
import concourse.bass as bass
import concourse.mybir as mybir
import concourse.tile as tile
from bass_rust import ScopedClock, SemaphoreHandle
from concourse.bass_utils import run_bass_kernel_spmd
from concourse.masks import make_identity

# ---------------------------------------------------------------------------
# Workaround for walrus codegen "Too many sync wait commands" on the
# TileContext tail drain: the CoreV3 CTRL encoding takes one sync wait, so
# replay the drain's wait set as standalone single-wait SP instructions.
# ---------------------------------------------------------------------------


def _drain_and_barrier_split(self, tick_clock, wait_clock):
    nc = self.nc
    probe = nc.sync.nop()
    wait_clock.add_sem_waits(probe.ins, ScopedClock({None: tick_clock.global_clock}))
    si = probe.ins.sync_info
    waits = list(si.on_wait or []) if si is not None else []
    if si is not None:
        si.on_wait = []
        probe.ins.sync_info = si
    for w in waits:
        op = {"sem-ge-imm": "sem-ge", "sem-eq-imm": "sem-eq"}.get(w.wait_mode, "sem-ge")
        nc.sync.wait_op(SemaphoreHandle(w.ant_name or "w", w.id), w.wait_value, op)
    nc.sync.drain()

    nc.all_engine_barrier()
    assert self.sems is not None
    popped = nc._tile_sem_poison_stack.pop()
    assert popped is self._sem_poison
    nc.clear_and_free_semaphores(list(self.sems.allocated().values()))
    nc.all_engine_barrier()


tile.TileContext._drain_and_barrier = _drain_and_barrier_split


def legalize_waits(nc, max_waits=1):
    """Walrus's ISA structs encode at most one sync wait per instruction.
    Hoist extra waits onto standalone same-engine NOPs inserted right
    before the over-subscribed instruction (identical blocking semantics)."""
    cur_list = nc.cur_bb.bb.instructions
    for bb in nc.m.functions[0].blocks:
        insts = bb.instructions
        i = 0
        while i < len(insts):
            ins = insts[i]
            si = getattr(ins, "sync_info", None)
            waits = list(si.on_wait or []) if si is not None else []
            movable = [w for w in waits if w.wait_reg is None]
            if len(waits) > max_waits and len(movable) > len(waits) - max_waits:
                nkeep = max_waits
                extra = movable[: len(waits) - nkeep]
                extra_set = {id(w) for w in extra}
                si.on_wait = [w for w in waits if id(w) not in extra_set]
                ins.sync_info = si
                carriers = []
                for w in extra:
                    nop = nc.engines[ins.engine].nop().ins
                    # the builder appended it to cur_bb; move it here
                    popped = cur_list.pop()
                    assert popped is nop
                    nop.sync_info = bass_rust.SyncInfo(on_wait=[w], on_update=[])
                    carriers.append(nop)
                insts[i:i] = carriers
                i += len(carriers)
            i += 1

# ---------------------------------------------------------------------------

B, T, E, H = 4, 2048, 64, 8
NCORES = 8
QS = T // NCORES          # 256 queries per core
NT = B * T // 128         # 64 token tiles of 128
KTB = T // 128            # 16 key tiles per batch
SCALE = 1.0 / np.sqrt(E)  # folded into exp()

F32 = mybir.dt.float32
F32R = mybir.dt.float32r
BF16 = mybir.dt.bfloat16
I32 = mybir.dt.int32

Exp = mybir.ActivationFunctionType.Exp


def build():
    nc = bass.Bass()
    x_d = nc.dram_tensor("x", [B * T, E], F32, kind="ExternalInput")
    y_d = nc.dram_tensor("ysl", [B * QS, E], F32, kind="ExternalInput")
    m_d = nc.dram_tensor("masksl", [QS, T], I32, kind="ExternalInput")
    wk_d = nc.dram_tensor("Wk", [E, E * H], F32, kind="ExternalInput")
    wq_d = nc.dram_tensor("Wq", [E, E * H], F32, kind="ExternalInput")
    wv_d = nc.dram_tensor("Wv", [E, E * H], F32, kind="ExternalInput")
    wu_d = nc.dram_tensor("Wu", [E * H, E], F32, kind="ExternalInput")
    bu_d = nc.dram_tensor("bu", [1, E], F32, kind="ExternalInput")
    out_d = nc.dram_tensor("out", [B * QS, E], F32, kind="ExternalOutput")

    with tile.TileContext(nc) as tc:
        with (
            tc.tile_pool(name="const", bufs=1) as cp,
            tc.tile_pool(name="big", bufs=1) as bigp,
            tc.tile_pool(name="att", bufs=4) as attp,
            tc.tile_pool(name="small", bufs=2) as smallp,
            tc.tile_pool(name="qp", bufs=2) as qpool,
            tc.tile_pool(name="ps512", bufs=2, space="PSUM") as pss,
            tc.tile_pool(name="pso", bufs=3, space="PSUM") as pso,
            tc.tile_pool(name="psb", bufs=1, space="PSUM") as psb,
        ):
            # ---- constants ----
            ident = cp.tile([128, 128], F32)
            make_identity(nc, ident[:])
            wk_t = cp.tile([64, 512], F32)
            nc.sync.dma_start(wk_t[:], wk_d[:])
            wq_t = cp.tile([64, 512], F32)
            nc.sync.dma_start(wq_t[:], wq_d[:])
            wv_t = cp.tile([64, 512], F32)
            wu8 = cp.tile([64, H, 64], F32)
            bu_f = cp.tile([1, 64], F32)
            bub = cp.tile([1, 64], BF16)
            ones_r = cp.tile([1, 128], BF16)   # bias broadcast lhsT
            nc.gpsimd.memset(ones_r[:], 1.0)
            ones_c = cp.tile([1, 64], F32)     # denom broadcast lhsT
            nc.gpsimd.memset(ones_c[:], 1.0)

            # ---- persistent big tiles ----
            xT = bigp.tile([64, B * T], F32R)     # x^T [e_in, tokens]
            yT = bigp.tile([64, B * QS], F32R)    # y^T [e_in, queries]
            mT = bigp.tile([128, KTB, QS], BF16)  # mask^T [k, q] per k-tile
            Xb = bigp.tile([128, NT, 65], BF16)   # X|1 per token tile
            W3T = bigp.tile([64, 4, 128], F32R)   # (Wk_h Wq_h^T)^T head pairs
            M2 = bigp.tile([64, H, 64], BF16)     # Wv_h @ Wu_h
            Obn = bigp.tile([64, B, H, QS], BF16)  # normalized O'^T
            outs = bigp.tile([128, 8, 64], F32)   # final output staging

            nc.gpsimd.memset(Xb[:, :, 64:65], 1.0)  # ones column

            # ---- setup: transposes, weight products ----
            with (
                tc.tile_pool(name="ldx", bufs=1) as ldp,
                tc.tile_pool(name="ldx2", bufs=3) as ldp2,
                tc.tile_pool(name="ldm", bufs=1) as lmp,
            ):
                # y^T from one staged load
                ys = ldp.tile([128, B * QS // 128, 64], F32, tag="ys")
                nc.sync.dma_start(
                    ys[:], y_d[:].rearrange("(n p) e -> p n e", p=128)
                )
                for i in range(B * QS // 128):
                    pt = pso.tile([128, 128], F32, tag="pO")
                    nc.tensor.transpose(pt[:64, :], ys[:, i, :], ident[:])
                    nc.vector.tensor_copy(yT[:, i * 128:(i + 1) * 128], pt[:64, :])

                # per-head weight transposes (f32, tiny)
                wkT = cp.tile([64, H, 64], F32)
                wqT = cp.tile([64, H, 64], F32)
                wvT = cp.tile([64, H, 64], F32)
                for h in range(H):
                    for wsrc, wdst in ((wk_t, wkT), (wq_t, wqT)):
                        pt = psb.tile([64, 64], F32, tag="pB")
                        nc.tensor.transpose(
                            pt[:], wsrc[:, h * 64:(h + 1) * 64], ident[:64, :64]
                        )
                        nc.vector.tensor_copy(wdst[:, h, :], pt[:])
                # W3_h^T = Wq_h^T . (Wk_h^T as rhs): out [ein_q, ein_k]
                for h in range(H):
                    pt = psb.tile([64, 64], F32, tag="pB")
                    nc.tensor.matmul(
                        pt[:], wqT[:, h, :], wkT[:, h, :], start=True, stop=True
                    )
                    nc.vector.tensor_copy(
                        W3T[:, h // 2, (h % 2) * 64:(h % 2) * 64 + 64], pt[:]
                    )
                # mask: int32 -> f32 -> transpose -> bf16 m^T
                for qt in range(QS // 128):
                    mi = lmp.tile([128, T], I32, tag="mi")
                    for mh in range(2):
                        nc.sync.dma_start(
                            mi[:, mh * 1024:(mh + 1) * 1024],
                            m_d[qt * 128:(qt + 1) * 128,
                                mh * 1024:(mh + 1) * 1024],
                        )
                    mf = lmp.tile([128, T], F32, tag="mf")
                    for mh in range(2):
                        nc.gpsimd.tensor_copy(
                            mf[:, mh * 1024:(mh + 1) * 1024],
                            mi[:, mh * 1024:(mh + 1) * 1024],
                        )
                    for ki in range(KTB):
                        pt = pso.tile([128, 128], F32, tag="pO")
                        nc.tensor.transpose(
                            pt[:], mf[:, ki * 128:(ki + 1) * 128], ident[:]
                        )
                        nc.scalar.copy(mT[:, ki, qt * 128:(qt + 1) * 128], pt[:])

                # x^T (f32r) and X|1 (bf16), streamed in 8 chunks so the
                # first head pair can start before the whole load finishes
                NX = 8
                for c in range(NT // NX):
                    xs = ldp2.tile([128, NX, 64], F32, tag="xs")
                    nc.sync.dma_start(
                        xs[:],
                        x_d[c * NX * 128:(c + 1) * NX * 128, :].rearrange(
                            "(n p) e -> p n e", p=128
                        ),
                    )
                    for j in range(NX):
                        i = c * NX + j
                        pt = pso.tile([128, 128], F32, tag="pO")
                        nc.tensor.transpose(pt[:64, :], xs[:, j, :], ident[:])
                        nc.vector.tensor_copy(
                            xT[:, i * 128:(i + 1) * 128], pt[:64, :]
                        )
                        nc.gpsimd.tensor_copy(Xb[:, i, 0:64], xs[:, j, :])

                # late-needed weights (M2 inputs + bias), after the
                # latency-critical loads have their DMA triggers queued
                nc.sync.dma_start(wv_t[:], wv_d[:])
                for h in range(H):
                    nc.sync.dma_start(wu8[:, h, :], wu_d[h * 64:(h + 1) * 64, :])
                nc.sync.dma_start(bu_f[:], bu_d[:])
                nc.vector.tensor_copy(bub[:], bu_f[:])

                for h in range(H):
                    pt = psb.tile([64, 64], F32, tag="pB")
                    nc.tensor.transpose(
                        pt[:], wv_t[:, h * 64:(h + 1) * 64], ident[:64, :64]
                    )
                    nc.vector.tensor_copy(wvT[:, h, :], pt[:])
                # M2_h = Wv_h @ Wu_h
                for h in range(H):
                    pt = psb.tile([64, 64], F32, tag="pB")
                    nc.tensor.matmul(
                        pt[:], wvT[:, h, :], wu8[:, h, :],
                        start=True, stop=True,
                    )
                    nc.vector.tensor_copy(M2[:, h, :], pt[:])


            # ---- main loop over head pairs ----
            for hp in range(4):
                # Q'^T for the pair: rows 0:64 = h0, 64:128 = h1
                QpT0 = qpool.tile([64, B * QS], F32R, tag="qp0")
                QpT1 = qpool.tile([64, B * QS], F32R, tag="qp1")
                for i in range(B * QS // 512):
                    pq = pss.tile([128, 1024], F32, tag="ps512")
                    nc.tensor.matmul(
                        pq[:, 0:512], W3T[:, hp, :],
                        yT[:, i * 512:(i + 1) * 512],
                        start=True, stop=True,
                    )
                    nc.vector.tensor_copy(
                        QpT0[:, i * 512:(i + 1) * 512], pq[0:64, 0:512]
                    )
                    nc.vector.tensor_copy(
                        QpT1[:, i * 512:(i + 1) * 512], pq[64:128, 0:512]
                    )

                for b in range(B):
                    for hh in range(2):
                        h = 2 * hp + hh
                        Qp = QpT0 if hh == 0 else QpT1
                        att = attp.tile([128, KTB, QS], BF16, tag="att")
                        pO = pso.tile([128, QS], F32, tag="pO")
                        for kj in range(KTB // 4):
                            gt = b * KTB + 4 * kj
                            pS = pss.tile([128, 1024], F32, tag="ps512")
                            for u in range(4):
                                nc.tensor.matmul(
                                    pS[:, u * 256:(u + 1) * 256],
                                    xT[:, (gt + u) * 128:(gt + u + 1) * 128],
                                    Qp[:, b * QS:(b + 1) * QS],
                                    start=True, stop=True,
                                )
                            nc.scalar.activation(
                                att[:, 4 * kj:4 * kj + 4, :], pS[:],
                                Exp, scale=SCALE,
                            )
                            nc.vector.tensor_mul(
                                att[:, 4 * kj:4 * kj + 4, :],
                                att[:, 4 * kj:4 * kj + 4, :],
                                mT[:, 4 * kj:4 * kj + 4, :],
                            )
                        for ki in range(KTB):
                            gt = b * KTB + ki
                            nc.tensor.matmul(
                                pO[0:65, :],
                                Xb[:, gt, :],
                                att[:, ki, :],
                                start=(ki == 0), stop=(ki == KTB - 1),
                                skip_group_check=True,
                            )
                        # normalize by the ones-column sums
                        rd = smallp.tile([1, QS], F32, tag="rd")
                        nc.vector.reciprocal(rd[:], pO[64:65, :])
                        pB = psb.tile([64, QS], F32, tag="pB")
                        nc.tensor.matmul(
                            pB[:], ones_c[:], rd[:], start=True, stop=True
                        )
                        pBc = smallp.tile([64, QS], F32, tag="pBc")
                        nc.vector.tensor_copy(pBc[:], pB[:])
                        nc.vector.tensor_mul(
                            Obn[:, b, h, :], pO[0:64, :], pBc[:]
                        )

            # ---- final: out = sum_h O'n_h^T M2_h + bu ----
            for b in range(B):
                for qt in range(QS // 128):
                    pU = pso.tile([128, 64], F32, tag="pO")
                    for h in range(H):
                        nc.tensor.matmul(
                            pU[:],
                            Obn[:, b, h, qt * 128:(qt + 1) * 128],
                            M2[:, h, :],
                            start=(h == 0), stop=False,
                            skip_group_check=True,
                        )
                    nc.tensor.matmul(
                        pU[:], ones_r[:], bub[:],
                        start=False, stop=True,
                        skip_group_check=True,
                    )
                    nc.scalar.copy(outs[:, b * 2 + qt, :], pU[:])
            nc.sync.dma_start(
                out_d[:].rearrange("(s p) e -> p s e", p=128), outs[:]
            )
    legalize_waits(nc)
    return nc


_NC = None


def _get_nc():
    global _NC
    if _NC is None:
        _NC = build()
    return _NC


LAST_EXEC_NS = None
LAST_RESULTS = None


def kernel(x, y, mask, Wk, Wq, Wv, Wu, bu, trace=False):
    global LAST_EXEC_NS, LAST_RESULTS
    x = np.ascontiguousarray(np.asarray(x, dtype=np.float32)).reshape(B * T, E)
    y = np.ascontiguousarray(np.asarray(y, dtype=np.float32))
    mask = np.ascontiguousarray(np.asarray(mask, dtype=np.int32))
    Wk = np.ascontiguousarray(np.asarray(Wk, dtype=np.float32))
    Wq = np.ascontiguousarray(np.asarray(Wq, dtype=np.float32))
    Wv = np.ascontiguousarray(np.asarray(Wv, dtype=np.float32))
    Wu = np.ascontiguousarray(np.asarray(Wu, dtype=np.float32))
    bu = np.ascontiguousarray(np.asarray(bu, dtype=np.float32)).reshape(1, E)

    nc = _get_nc()
    in_maps = []
    for c in range(NCORES):
        q0 = c * QS
        in_maps.append({
            "x": x,
            "ysl": np.ascontiguousarray(y[:, q0:q0 + QS, :]).reshape(B * QS, E),
            "masksl": np.ascontiguousarray(mask[0, q0:q0 + QS, :]),
            "Wk": Wk, "Wq": Wq, "Wv": Wv, "Wu": Wu, "bu": bu,
        })
    res = run_bass_kernel_spmd(
        nc, in_maps, core_ids=list(range(NCORES)), trace=trace
    )
    LAST_EXEC_NS = res.exec_time_ns
    LAST_RESULTS = res
    out = np.empty((B, T, E), dtype=np.float32)
    for c in range(NCORES):
        q0 = c * QS
        out[:, q0:q0 + QS, :] = res.results[c]["out"].reshape(B, QS, E)
    return out


# revision 28
# speedup vs baseline: 1.0222x; 1.0007x over previous
"""CrossAttention kernel for 8 Trainium2 NeuronCores.

Problem: x,y [4,2048,64] f32, mask [1,2048,2048] i32, per-head projections
Wk/Wq/Wv [64, 512] (8 heads x head_dim 64), unify Wu [512,64] + bu.

Sharding: split the query axis t_y across the 8 cores (256 queries each, for
all batches/heads). Each core reads the full x (2MB), its y slice, and its
256x2048 mask slice; writes a disjoint out slice. No collectives needed.

Per-core layout: scores are computed transposed (S^T [k, q]) so that the
attention*V matmul can consume them directly as the moving operand with V as
the stationary operand, producing O^T [e, q] chunks that feed the unify GEMM
as its stationary operand with no further transposes. Softmax over k (the
partition dim of S^T) needs no max-subtraction (|S|<~1 by construction); the
denominator comes for free from a ones-column appended to V, and the
binary mask is applied multiplicatively to exp(S^T) on the vector engine.
"""

import numpy as np

import bass_rust
import concourse.bass as bass
import concourse.mybir as mybir
import concourse.tile as tile
from bass_rust import ScopedClock, SemaphoreHandle
from concourse.bass_utils import run_bass_kernel_spmd
from concourse.masks import make_identity

# ---------------------------------------------------------------------------
# Workaround for walrus codegen "Too many sync wait commands" on the
# TileContext tail drain: the CoreV3 CTRL encoding takes one sync wait, so
# replay the drain's wait set as standalone single-wait SP instructions.
# ---------------------------------------------------------------------------


def _drain_and_barrier_split(self, tick_clock, wait_clock):
    nc = self.nc
    probe = nc.sync.nop()
    wait_clock.add_sem_waits(probe.ins, ScopedClock({None: tick_clock.global_clock}))
    si = probe.ins.sync_info
    waits = list(si.on_wait or []) if si is not None else []
    if si is not None:
        si.on_wait = []
        probe.ins.sync_info = si
    for w in waits:
        op = {"sem-ge-imm": "sem-ge", "sem-eq-imm": "sem-eq"}.get(w.wait_mode, "sem-ge")
        nc.sync.wait_op(SemaphoreHandle(w.ant_name or "w", w.id), w.wait_value, op)
    nc.sync.drain()

    nc.all_engine_barrier()
    assert self.sems is not None
    popped = nc._tile_sem_poison_stack.pop()
    assert popped is self._sem_poison
    nc.clear_and_free_semaphores(list(self.sems.allocated().values()))
    nc.all_engine_barrier()


tile.TileContext._drain_and_barrier = _drain_and_barrier_split


def legalize_waits(nc, max_waits=1):
    """Walrus's ISA structs encode at most one sync wait per instruction.
    Hoist extra waits onto standalone same-engine NOPs inserted right
    before the over-subscribed instruction (identical blocking semantics)."""
    cur_list = nc.cur_bb.bb.instructions
    for bb in nc.m.functions[0].blocks:
        insts = bb.instructions
        i = 0
        while i < len(insts):
            ins = insts[i]
            si = getattr(ins, "sync_info", None)
            waits = list(si.on_wait or []) if si is not None else []
            movable = [w for w in waits if w.wait_reg is None]
            if len(waits) > max_waits and len(movable) > len(waits) - max_waits:
                nkeep = max_waits
                extra = movable[: len(waits) - nkeep]
                extra_set = {id(w) for w in extra}
                si.on_wait = [w for w in waits if id(w) not in extra_set]
                ins.sync_info = si
                carriers = []
                for w in extra:
                    nop = nc.engines[ins.engine].nop().ins
                    # the builder appended it to cur_bb; move it here
                    popped = cur_list.pop()
                    assert popped is nop
                    nop.sync_info = bass_rust.SyncInfo(on_wait=[w], on_update=[])
                    carriers.append(nop)
                insts[i:i] = carriers
                i += len(carriers)
            i += 1

# ---------------------------------------------------------------------------

B, T, E, H = 4, 2048, 64, 8
NCORES = 8
QS = T // NCORES          # 256 queries per core
NT = B * T // 128         # 64 token tiles of 128
KTB = T // 128            # 16 key tiles per batch
SCALE = 1.0 / np.sqrt(E)  # folded into exp()

F32 = mybir.dt.float32
F32R = mybir.dt.float32r
BF16 = mybir.dt.bfloat16
I32 = mybir.dt.int32

Exp = mybir.ActivationFunctionType.Exp


def build():
    nc = bass.Bass()
    x_d = nc.dram_tensor("x", [B * T, E], F32, kind="ExternalInput")
    y_d = nc.dram_tensor("ysl", [B * QS, E], F32, kind="ExternalInput")
    m_d = nc.dram_tensor("masksl", [QS, T], I32, kind="ExternalInput")
    wk_d = nc.dram_tensor("Wk", [E, E * H], F32, kind="ExternalInput")
    wq_d = nc.dram_tensor("Wq", [E, E * H], F32, kind="ExternalInput")
    wv_d = nc.dram_tensor("Wv", [E, E * H], F32, kind="ExternalInput")
    wu_d = nc.dram_tensor("Wu", [E * H, E], F32, kind="ExternalInput")
    bu_d = nc.dram_tensor("bu", [1, E], F32, kind="ExternalInput")
    out_d = nc.dram_tensor("out", [B * QS, E], F32, kind="ExternalOutput")

    with tile.TileContext(nc) as tc:
        with (
            tc.tile_pool(name="const", bufs=1) as cp,
            tc.tile_pool(name="big", bufs=1) as bigp,
            tc.tile_pool(name="att", bufs=4) as attp,
            tc.tile_pool(name="small", bufs=2) as smallp,
            tc.tile_pool(name="qp", bufs=2) as qpool,
            tc.tile_pool(name="ps512", bufs=2, space="PSUM") as pss,
            tc.tile_pool(name="pso", bufs=3, space="PSUM") as pso,
            tc.tile_pool(name="psb", bufs=1, space="PSUM") as psb,
        ):
            # ---- constants ----
            ident = cp.tile([128, 128], F32)
            make_identity(nc, ident[:])
            wk_t = cp.tile([64, 512], F32)
            nc.sync.dma_start(wk_t[:], wk_d[:])
            wq_t = cp.tile([64, 512], F32)
            nc.sync.dma_start(wq_t[:], wq_d[:])
            wv_t = cp.tile([64, 512], F32)
            wu8 = cp.tile([64, H, 64], F32)
            bu_f = cp.tile([1, 64], F32)
            bub = cp.tile([1, 64], BF16)
            ones_r = cp.tile([1, 128], BF16)   # bias broadcast lhsT
            nc.gpsimd.memset(ones_r[:], 1.0)
            ones_c = cp.tile([1, 64], F32)     # denom broadcast lhsT
            nc.gpsimd.memset(ones_c[:], 1.0)

            # ---- persistent big tiles ----
            xT = bigp.tile([64, B * T], F32R)     # x^T [e_in, tokens]
            yT = bigp.tile([64, B * QS], F32R)    # y^T [e_in, queries]
            mT = bigp.tile([128, KTB, QS], BF16)  # mask^T [k, q] per k-tile
            Xb = bigp.tile([128, NT, 65], BF16)   # X|1 per token tile
            W3T = bigp.tile([64, 4, 128], F32R)   # (Wk_h Wq_h^T)^T head pairs
            M2 = bigp.tile([64, H, 64], BF16)     # Wv_h @ Wu_h
            Obn = bigp.tile([64, B, H, QS], BF16)  # normalized O'^T
            outs = bigp.tile([128, 8, 64], F32)   # final output staging

            nc.gpsimd.memset(Xb[:, :, 64:65], 1.0)  # ones column

            # ---- setup: transposes, weight products ----
            with (
                tc.tile_pool(name="ldx", bufs=1) as ldp,
                tc.tile_pool(name="ldx2", bufs=3) as ldp2,
                tc.tile_pool(name="ldm", bufs=1) as lmp,
            ):
                # y^T from one staged load
                ys = ldp.tile([128, B * QS // 128, 64], F32, tag="ys")
                nc.sync.dma_start(
                    ys[:], y_d[:].rearrange("(n p) e -> p n e", p=128)
                )
                for i in range(B * QS // 128):
                    pt = pso.tile([128, 128], F32, tag="pO")
                    nc.tensor.transpose(pt[:64, :], ys[:, i, :], ident[:])
                    nc.vector.tensor_copy(yT[:, i * 128:(i + 1) * 128], pt[:64, :])

                # per-head weight transposes (f32, tiny)
                wkT = cp.tile([64, H, 64], F32)
                wqT = cp.tile([64, H, 64], F32)
                wvT = cp.tile([64, H, 64], F32)
                for h in range(H):
                    for wsrc, wdst in ((wk_t, wkT), (wq_t, wqT)):
                        pt = psb.tile([64, 64], F32, tag="pB")
                        nc.tensor.transpose(
                            pt[:], wsrc[:, h * 64:(h + 1) * 64], ident[:64, :64]
                        )
                        nc.vector.tensor_copy(wdst[:, h, :], pt[:])
                # W3_h^T = Wq_h^T . (Wk_h^T as rhs): out [ein_q, ein_k]
                for h in range(H):
                    pt = psb.tile([64, 64], F32, tag="pB")
                    nc.tensor.matmul(
                        pt[:], wqT[:, h, :], wkT[:, h, :], start=True, stop=True
                    )
                    nc.vector.tensor_copy(
                        W3T[:, h // 2, (h % 2) * 64:(h % 2) * 64 + 64], pt[:]
                    )
                # mask: int32 -> f32 -> transpose -> bf16 m^T
                for qt in range(QS // 128):
                    mi = lmp.tile([128, T], I32, tag="mi")
                    for mh in range(2):
                        nc.sync.dma_start(
                            mi[:, mh * 1024:(mh + 1) * 1024],
                            m_d[qt * 128:(qt + 1) * 128,
                                mh * 1024:(mh + 1) * 1024],
                        )
                    mf = lmp.tile([128, T], F32, tag="mf")
                    for mh in range(2):
                        nc.gpsimd.tensor_copy(
                            mf[:, mh * 1024:(mh + 1) * 1024],
                            mi[:, mh * 1024:(mh + 1) * 1024],
                        )
                    for ki in range(KTB):
                        pt = pso.tile([128, 128], F32, tag="pO")
                        nc.tensor.transpose(
                            pt[:], mf[:, ki * 128:(ki + 1) * 128], ident[:]
                        )
                        nc.scalar.copy(mT[:, ki, qt * 128:(qt + 1) * 128], pt[:])

                # x^T (f32r) and X|1 (bf16), streamed in 8 chunks so the
                # first head pair can start before the whole load finishes
                NX = 8
                for c in range(NT // NX):
                    xs = ldp2.tile([128, NX, 64], F32, tag="xs")
                    nc.sync.dma_start(
                        xs[:],
                        x_d[c * NX * 128:(c + 1) * NX * 128, :].rearrange(
                            "(n p) e -> p n e", p=128
                        ),
                    )
                    for j in range(NX):
                        i = c * NX + j
                        pt = pso.tile([128, 128], F32, tag="pO")
                        nc.tensor.transpose(pt[:64, :], xs[:, j, :], ident[:])
                        nc.vector.tensor_copy(
                            xT[:, i * 128:(i + 1) * 128], pt[:64, :]
                        )
                        nc.gpsimd.tensor_copy(Xb[:, i, 0:64], xs[:, j, :])

                # late-needed weights (M2 inputs + bias), after the
                # latency-critical loads have their DMA triggers queued
                nc.sync.dma_start(wv_t[:], wv_d[:])
                for h in range(H):
                    nc.sync.dma_start(wu8[:, h, :], wu_d[h * 64:(h + 1) * 64, :])
                nc.sync.dma_start(bu_f[:], bu_d[:])
                nc.vector.tensor_copy(bub[:], bu_f[:])

                for h in range(H):
                    pt = psb.tile([64, 64], F32, tag="pB")
                    nc.tensor.transpose(
                        pt[:], wv_t[:, h * 64:(h + 1) * 64], ident[:64, :64]
                    )
                    nc.vector.tensor_copy(wvT[:, h, :], pt[:])
                # M2_h = Wv_h @ Wu_h
                for h in range(H):
                    pt = psb.tile([64, 64], F32, tag="pB")
                    nc.tensor.matmul(
                        pt[:], wvT[:, h, :], wu8[:, h, :],
                        start=True, stop=True,
                    )
                    nc.vector.tensor_copy(M2[:, h, :], pt[:])


            # ---- main loop over head pairs ----
            for hp in range(4):
                # Q'^T for the pair: rows 0:64 = h0, 64:128 = h1
                QpT0 = qpool.tile([64, B * QS], F32R, tag="qp0")
                QpT1 = qpool.tile([64, B * QS], F32R, tag="qp1")
                for i in range(B * QS // 512):
                    pq = pss.tile([128, 1024], F32, tag="ps512")
                    nc.tensor.matmul(
                        pq[:, 0:512], W3T[:, hp, :],
                        yT[:, i * 512:(i + 1) * 512],
                        start=True, stop=True,
                    )
                    nc.vector.tensor_copy(
                        QpT0[:, i * 512:(i + 1) * 512], pq[0:64, 0:512]
                    )
                    nc.scalar.copy(
                        QpT1[:, i * 512:(i + 1) * 512], pq[64:128, 0:512]
                    )

                for b in range(B):
                    for hh in range(2):
                        h = 2 * hp + hh
                        Qp = QpT0 if hh == 0 else QpT1
                        att = attp.tile([128, KTB, QS], BF16, tag="att")
                        pO = pso.tile([128, QS], F32, tag="pO")
                        for kj in range(KTB // 4):
                            gt = b * KTB + 4 * kj
                            pS = pss.tile([128, 1024], F32, tag="ps512")
                            for u in range(4):
                                nc.tensor.matmul(
                                    pS[:, u * 256:(u + 1) * 256],
                                    xT[:, (gt + u) * 128:(gt + u + 1) * 128],
                                    Qp[:, b * QS:(b + 1) * QS],
                                    start=True, stop=True,
                                )
                            nc.scalar.activation(
                                att[:, 4 * kj:4 * kj + 4, :], pS[:],
                                Exp, scale=SCALE,
                            )
                            nc.vector.tensor_mul(
                                att[:, 4 * kj:4 * kj + 4, :],
                                att[:, 4 * kj:4 * kj + 4, :],
                                mT[:, 4 * kj:4 * kj + 4, :],
                            )
                        for ki in range(KTB):
                            gt = b * KTB + ki
                            nc.tensor.matmul(
                                pO[0:65, :],
                                Xb[:, gt, :],
                                att[:, ki, :],
                                start=(ki == 0), stop=(ki == KTB - 1),
                                skip_group_check=True,
                            )
                        # normalize by the ones-column sums
                        rd = smallp.tile([1, QS], F32, tag="rd")
                        nc.vector.reciprocal(rd[:], pO[64:65, :])
                        pB = psb.tile([64, QS], F32, tag="pB")
                        nc.tensor.matmul(
                            pB[:], ones_c[:], rd[:], start=True, stop=True
                        )
                        pBc = smallp.tile([64, QS], F32, tag="pBc")
                        nc.scalar.copy(pBc[:], pB[:])
                        nc.vector.tensor_mul(
                            Obn[:, b, h, :], pO[0:64, :], pBc[:]
                        )

            # ---- final: out = sum_h O'n_h^T M2_h + bu ----
            for b in range(B):
                for qt in range(QS // 128):
                    pU = pso.tile([128, 64], F32, tag="pO")
                    for h in range(H):
                        nc.tensor.matmul(
                            pU[:],
                            Obn[:, b, h, qt * 128:(qt + 1) * 128],
                            M2[:, h, :],
                            start=(h == 0), stop=False,
                            skip_group_check=True,
                        )
                    nc.tensor.matmul(
                        pU[:], ones_r[:], bub[:],
                        start=False, stop=True,
                        skip_group_check=True,
                    )
                    nc.scalar.copy(outs[:, b * 2 + qt, :], pU[:])
            nc.sync.dma_start(
                out_d[:].rearrange("(s p) e -> p s e", p=128), outs[:]
            )
    legalize_waits(nc)
    return nc


_NC = None


def _get_nc():
    global _NC
    if _NC is None:
        _NC = build()
    return _NC


LAST_EXEC_NS = None
LAST_RESULTS = None


def kernel(x, y, mask, Wk, Wq, Wv, Wu, bu, trace=False):
    global LAST_EXEC_NS, LAST_RESULTS
    x = np.ascontiguousarray(np.asarray(x, dtype=np.float32)).reshape(B * T, E)
    y = np.ascontiguousarray(np.asarray(y, dtype=np.float32))
    mask = np.ascontiguousarray(np.asarray(mask, dtype=np.int32))
    Wk = np.ascontiguousarray(np.asarray(Wk, dtype=np.float32))
    Wq = np.ascontiguousarray(np.asarray(Wq, dtype=np.float32))
    Wv = np.ascontiguousarray(np.asarray(Wv, dtype=np.float32))
    Wu = np.ascontiguousarray(np.asarray(Wu, dtype=np.float32))
    bu = np.ascontiguousarray(np.asarray(bu, dtype=np.float32)).reshape(1, E)

    nc = _get_nc()
    in_maps = []
    for c in range(NCORES):
        q0 = c * QS
        in_maps.append({
            "x": x,
            "ysl": np.ascontiguousarray(y[:, q0:q0 + QS, :]).reshape(B * QS, E),
            "masksl": np.ascontiguousarray(mask[0, q0:q0 + QS, :]),
            "Wk": Wk, "Wq": Wq, "Wv": Wv, "Wu": Wu, "bu": bu,
        })
    res = run_bass_kernel_spmd(
        nc, in_maps, core_ids=list(range(NCORES)), trace=trace
    )
    LAST_EXEC_NS = res.exec_time_ns
    LAST_RESULTS = res
    out = np.empty((B, T, E), dtype=np.float32)
    for c in range(NCORES):
        q0 = c * QS
        out[:, q0:q0 + QS, :] = res.results[c]["out"].reshape(B, QS, E)
    return out
